# revision 6
# baseline (speedup 1.0000x reference)
import os
import sys

sys.path.insert(0, "/opt/trn_rl_repo")

import numpy as np
import ml_dtypes
import bass_rust
from concourse import bass, mybir
from concourse.tile import TileContext
from concourse.vector_clock import ScopedClock
from concourse.bass_utils import run_bass_kernel_spmd

B, S, E, H = 4, 2048, 1024, 1024
NCORES = 8
EC = E // 128  # contraction chunks
HC = H // 128  # h chunks
KC = S // 128  # key chunks (16)
NCLS = 4  # tile classes per core; each class owns 2 query tiles
F32 = mybir.dt.float32
BF = mybir.dt.bfloat16
BFNP = ml_dtypes.bfloat16

# Results of the last run_bass_kernel_spmd call (for test harness inspection).
LAST_RESULT = None


def _tile_map(parity):
    """(cls, half) -> global 128-row query tile index.

    Even cores take tiles {4c, 4c+3}, odd cores {4c+1, 4c+2}: both cores then
    process an identical padded chunk schedule (4c+2 both-tile chunks plus 2
    B-only chunks per class), with per-core mask data absorbing the
    difference.
    """
    m = {}
    for c in range(NCLS):
        if parity == 0:
            m[(c, 0)], m[(c, 1)] = 4 * c, 4 * c + 3
        else:
            m[(c, 0)], m[(c, 1)] = 4 * c + 1, 4 * c + 2
    return m


class PatchedTileContext(TileContext):
    """TileContext whose tail drain carries at most one sem wait.

    The walrus codegen in this container rejects a Drain with more than one
    sync wait ("Too many sync wait commands"); split the global-clock waits
    across a chain of drains on the same engine instead.
    """

    def _drain_and_barrier(self, tick_clock, wait_clock):
        drain_inst = self.nc.sync.drain()
        wait_clock.add_sem_waits(
            drain_inst.ins, ScopedClock({None: tick_clock.global_clock})
        )
        mi = drain_inst.ins
        waits = list(mi.sync_info.on_wait)
        ups = list(mi.sync_info.on_update)
        if len(waits) > 1:
            mi.sync_info = bass_rust.SyncInfo(on_wait=waits[:1], on_update=[])
            for i, w in enumerate(waits[1:]):
                last = i == len(waits) - 2
                d2 = self.nc.sync.drain()
                d2.ins.sync_info = bass_rust.SyncInfo(
                    on_wait=[w], on_update=ups if last else []
                )
        self.nc.all_engine_barrier()
        assert self.sems is not None
        popped = self.nc._tile_sem_poison_stack.pop()
        assert popped is self._sem_poison
        self.nc.clear_and_free_semaphores(list(self.sems.allocated().values()))
        self.nc.all_engine_barrier()


def _split_multi_waits(json_bytes):
    """Rewrite BIR so no instruction carries more than one sync wait."""
    import json as _json

    d = _json.loads(json_bytes)
    ctr = 0
    for f in d.get("functions", []):
        for blk in f.get("blocks", []):
            insts = blk.get("instructions", [])
            out = []
            for inst in insts:
                si = inst.get("sync_info") or {}
                ow = si.get("on_wait") or []
                if len(ow) > 1:
                    for w in ow[:-1]:
                        out.append(
                            {
                                "debug": inst.get("debug", 0),
                                "engine": inst["engine"],
                                "ins": [],
                                "name": f"wsplit_{ctr}",
                                "opcode": "NoOp",
                                "outs": [],
                                "sync_info": {"on_update": [], "on_wait": [w]},
                            }
                        )
                        ctr += 1
                    si = dict(si)
                    si["on_wait"] = [ow[-1]]
                    inst = dict(inst)
                    inst["sync_info"] = si
                out.append(inst)
            blk["instructions"] = out
    return _json.dumps(d).encode()


def _build_program():
    nc = bass.Bass("TRN2", target_bir_lowering=False, debug=False, num_devices=NCORES)
    orig_to_json_bytes = nc.to_json_bytes
    nc.to_json_bytes = lambda: _split_multi_waits(orig_to_json_bytes())

    xT = nc.dram_tensor("xT", [E, S], BF, kind="ExternalInput")
    xq = nc.dram_tensor("xq", [E, NCLS * 256], BF, kind="ExternalInput")
    wqT = nc.dram_tensor("wqT", [E, H], BF, kind="ExternalInput")
    wkT = nc.dram_tensor("wkT", [E, H], BF, kind="ExternalInput")
    wvT = nc.dram_tensor("wvT", [E, H], BF, kind="ExternalInput")
    bqs = nc.dram_tensor("bqs", [H], F32, kind="ExternalInput")
    bk = nc.dram_tensor("bk", [H], F32, kind="ExternalInput")
    bv = nc.dram_tensor("bv", [H], F32, kind="ExternalInput")
    msk = nc.dram_tensor("msk", [4, 128, 128], BF, kind="ExternalInput")
    out = nc.dram_tensor("out", [2 * NCLS, 128, H], F32, kind="ExternalOutput")

    with PatchedTileContext(nc) as tc:
        import os
        with (
            tc.tile_pool(name="const", bufs=1) as const_pool,
            tc.tile_pool(name="xp", bufs=1) as x_pool,
            tc.tile_pool(name="ktp", bufs=1) as kt_pool,
            tc.tile_pool(name="qtp", bufs=1) as qt_pool,
            tc.tile_pool(name="vp", bufs=1) as v_pool,
        ):
            bcst = const_pool.tile([128, 2 * HC], F32, tag="bcst")
            bq_t = bcst[:, 0:HC]
            bk_t = bcst[:, HC : 2 * HC]
            bvb = const_pool.tile([128, H], F32, tag="bvb")
            msk_sb = const_pool.tile([128, 4, 128], BF, tag="msk")
            ones_t = const_pool.tile([128, 1], BF, tag="ones")
            nc.vector.memset(ones_t, 1.0)
            nc.gpsimd.dma_start(out=bq_t, in_=bqs[:].rearrange("(c p) -> p c", p=128))
            nc.gpsimd.dma_start(out=bk_t, in_=bk[:].rearrange("(c p) -> p c", p=128))
            nc.gpsimd.dma_start(out=bvb, in_=bv[:].partition_broadcast(128))
            nc.gpsimd.dma_start(
                out=msk_sb, in_=msk[:, :, :].rearrange("m p q -> p m q")
            )

            x_sb = x_pool.tile([128, EC, S], BF, tag="x")
            kt = kt_pool.tile([128, HC, S], BF, tag="kt")
            qt = qt_pool.tile([128, HC, NCLS * 256], BF, tag="qt")
            v_sb = v_pool.tile([128, KC, H], BF, tag="v")

            with (
                tc.tile_pool(name="wkp", bufs=1) as wk_pool,
                tc.tile_pool(name="wqp", bufs=1) as wq_pool,
                tc.tile_pool(name="wvp", bufs=1) as wv_pool,
                tc.tile_pool(name="xqp", bufs=1) as xq_pool,
            ):
                wk_sb = wk_pool.tile([128, EC, H], BF, tag="wk")
                wq_sb = wq_pool.tile([128, EC, H], BF, tag="wq")
                wv_sb = wv_pool.tile([128, EC, H], BF, tag="wv")
                xq_sb = xq_pool.tile([128, EC, NCLS * 256], BF, tag="xq")

                # critical-path stream: wk/x chunk pairs feed the first matmuls
                for e in range(EC):
                    nc.sync.dma_start(
                        out=wk_sb[:, e, :], in_=wkT[e * 128 : (e + 1) * 128, :]
                    )
                    nc.sync.dma_start(
                        out=x_sb[:, e, :], in_=xT[e * 128 : (e + 1) * 128, :]
                    )
                # secondary stream
                for e in range(EC):
                    nc.gpsimd.dma_start(
                        out=xq_sb[:, e, :], in_=xq[e * 128 : (e + 1) * 128, :]
                    )
                for e in range(EC):
                    nc.gpsimd.dma_start(
                        out=wq_sb[:, e, :], in_=wqT[e * 128 : (e + 1) * 128, :]
                    )
                for e in range(EC):
                    nc.gpsimd.dma_start(
                        out=wv_sb[:, e, :], in_=wvT[e * 128 : (e + 1) * 128, :]
                    )

                # ---- K projection: kt[h, s] = (x @ Wk.T).T in bf16 ---------
                with tc.tile_pool(name="pskp", bufs=1, space="PSUM") as psk_pool:
                    for sl in range(4):
                        psk = [
                            psk_pool.tile([128, 512], F32, tag=f"psk{hc}", name=f"psk{hc}")
                            for hc in range(HC)
                        ]
                        for e in range(EC):
                            for hc in range(HC):
                                nc.tensor.matmul(
                                    psk[hc],
                                    lhsT=wk_sb[:, e, hc * 128 : (hc + 1) * 128],
                                    rhs=x_sb[:, e, sl * 512 : (sl + 1) * 512],
                                    start=(e == 0),
                                    stop=(e == EC - 1),
                                )
                        for hc in range(HC):
                            nc.vector.tensor_scalar_add(
                                kt[:, hc, sl * 512 : (sl + 1) * 512],
                                psk[hc],
                                bk_t[:, hc : hc + 1],
                            )

                # ---- Q projection (own tiles only) -------------------------
                with tc.tile_pool(name="psqp", bufs=1, space="PSUM") as psq_pool:
                    for c in range(NCLS):
                        psq = [
                            psq_pool.tile([128, 256], F32, tag=f"psq{hc}", name=f"psq{hc}")
                            for hc in range(HC)
                        ]
                        for e in range(EC):
                            for hc in range(HC):
                                nc.tensor.matmul(
                                    psq[hc],
                                    lhsT=wq_sb[:, e, hc * 128 : (hc + 1) * 128],
                                    rhs=xq_sb[:, e, c * 256 : (c + 1) * 256],
                                    start=(e == 0),
                                    stop=(e == EC - 1),
                                )
                        for hc in range(HC):
                            nc.vector.tensor_scalar_add(
                                qt[:, hc, c * 256 : (c + 1) * 256],
                                psq[hc],
                                bq_t[:, hc : hc + 1],
                            )

                # ---- V projection: v[s, h] per 128-row key chunk -----------
                with tc.tile_pool(name="psvp", bufs=2, space="PSUM") as psv_pool:
                    for kc in range(KC):
                        psv = [
                            psv_pool.tile([128, 512], F32, tag=f"psv{hh}", name=f"psv{hh}")
                            for hh in range(2)
                        ]
                        for e in range(EC):
                            for hh in range(2):
                                nc.tensor.matmul(
                                    psv[hh],
                                    lhsT=x_sb[:, e, kc * 128 : (kc + 1) * 128],
                                    rhs=wv_sb[:, e, hh * 512 : (hh + 1) * 512],
                                    start=(e == 0),
                                    stop=(e == EC - 1),
                                )
                        for hh in range(2):
                            nc.vector.tensor_add(
                                v_sb[:, kc, hh * 512 : (hh + 1) * 512],
                                psv[hh],
                                bvb[:, hh * 512 : (hh + 1) * 512],
                            )

            # ---- attention ------------------------------------------------
            if os.environ.get("K_SKIP_ATTN"):
                nc.gpsimd.dma_start(out=out[0, :, :], in_=kt[:, 0, 0:1024])
                return nc
            # S^T layout: scores land [key, query] in PSUM, exp writes P^T
            # straight to SBUF (no transposes, no max subtraction needed),
            # and the softmax denominator comes from a width-1 ones matmul
            # accumulated alongside the AV matmuls.
            with (
                tc.tile_pool(name="ptp", bufs=2) as pt_pool,
                tc.tile_pool(name="outp", bufs=2) as out_pool,
                tc.tile_pool(name="stat", bufs=4) as stat_pool,
                tc.tile_pool(name="spp", bufs=2, space="PSUM") as sp_pool,
                tc.tile_pool(name="pop", bufs=1, space="PSUM") as po_pool,
                tc.tile_pool(name="lp", bufs=1, space="PSUM") as l_pool,
            ):
                for c in range(NCLS):
                    nboth, ntot = 4 * c + 2, 4 * c + 4
                    pt = pt_pool.tile([128, KC, 256], BF, tag="pt", name=f"pt{c}")
                    po = [
                        [
                            po_pool.tile(
                                [128, 512], F32, tag=f"po{ht}{hh}", name=f"po{c}_{ht}{hh}"
                            )
                            for hh in range(2)
                        ]
                        for ht in range(2)
                    ]
                    lps = [
                        l_pool.tile([128, 1], F32, tag=f"l{ht}", name=f"l{c}_{ht}")
                        for ht in range(2)
                    ]

                    def scores(kc):
                        wide = kc < nboth
                        sp = sp_pool.tile([128, 256], F32, tag="sp", name=f"sp{c}_{kc}")
                        spo = sp if wide else sp[:, 0:128]
                        q0 = c * 256 if wide else c * 256 + 128
                        qw = 256 if wide else 128
                        for hc in range(HC):
                            nc.tensor.matmul(
                                spo,
                                lhsT=kt[:, hc, kc * 128 : (kc + 1) * 128],
                                rhs=qt[:, hc, q0 : q0 + qw],
                                start=(hc == 0),
                                stop=(hc == HC - 1),
                            )
                        dst = pt[:, kc, 0:256] if wide else pt[:, kc, 128:256]
                        if os.environ.get("K_SKIP_EXP"):
                            nc.vector.tensor_copy(dst, spo)
                        else:
                            nc.scalar.activation(
                                dst, spo, mybir.ActivationFunctionType.Exp
                            )
                        pos = kc - 4 * c
                        if 0 <= pos < 4:
                            ht = 0 if pos < 2 else 1
                            sl = pt[:, kc, ht * 128 : (ht + 1) * 128]
                            nc.vector.tensor_mul(sl, sl, msk_sb[:, pos, :])

                    def av(kc):
                        for ht in range(2):
                            last = nboth - 1 if ht == 0 else ntot - 1
                            if kc > last:
                                continue
                            lhsT = pt[:, kc, ht * 128 : (ht + 1) * 128]
                            for hh in range(2):
                                nc.tensor.matmul(
                                    po[ht][hh],
                                    lhsT=lhsT,
                                    rhs=v_sb[:, kc, hh * 512 : (hh + 1) * 512],
                                    start=(kc == 0),
                                    stop=(kc == last),
                                )
                            if not os.environ.get("K_SKIP_ONES"):
                                nc.tensor.matmul(
                                    lps[ht],
                                    lhsT=lhsT,
                                    rhs=ones_t,
                                    start=(kc == 0),
                                    stop=(kc == last),
                                )

                    for kc in range(ntot):
                        scores(kc)
                        if kc >= 2:
                            av(kc - 2)
                    av(ntot - 2)
                    av(ntot - 1)

                    for ht in range(2):
                        rl = stat_pool.tile([128, 1], F32, tag="rl", name=f"rl{c}_{ht}")
                        if os.environ.get("K_SKIP_ONES"):
                            nc.vector.memset(rl, 1.0)
                        else:
                            nc.vector.reciprocal(rl, lps[ht])
                        ot = out_pool.tile([128, H], F32, tag="ot", name=f"ot{c}_{ht}")
                        for hh in range(2):
                            nc.vector.tensor_scalar_mul(
                                ot[:, hh * 512 : (hh + 1) * 512], po[ht][hh], rl
                            )
                        nc.sync.dma_start(out=out[2 * c + ht, :, :], in_=ot)

    return nc


def kernel(inputs, Wq, bq, Wk, bk, Wv, bv):
    global LAST_RESULT
    inputs = np.ascontiguousarray(inputs, dtype=np.float32)
    scale = np.float32(1.0 / np.sqrt(np.float32(E)))

    wqT = np.ascontiguousarray((Wq.T.astype(np.float32) * scale)).astype(BFNP)
    wkT = np.ascontiguousarray(Wk.T.astype(np.float32)).astype(BFNP)
    wvT = np.ascontiguousarray(Wv.T.astype(np.float32)).astype(BFNP)
    bqs = (bq.astype(np.float32) * scale).copy()
    bk = np.ascontiguousarray(bk, dtype=np.float32)
    bv = np.ascontiguousarray(bv, dtype=np.float32)

    # mask pages: [A@4c, A@4c+1, B@4c+2, B@4c+3]; tri[k, q] = 1 iff k <= q
    kk = np.arange(128)[:, None]
    qq = np.arange(128)[None, :]
    tri = (kk <= qq).astype(np.float32)
    onesm = np.ones((128, 128), np.float32)
    zerom = np.zeros((128, 128), np.float32)
    msk_even = np.stack([tri, zerom, onesm, tri]).astype(BFNP)
    msk_odd = np.stack([onesm, tri, tri, zerom]).astype(BFNP)

    xTs = [np.ascontiguousarray(inputs[b].T).astype(BFNP) for b in range(B)]

    in_maps = []
    for core in range(NCORES):
        b, parity = core // 2, core % 2
        tm = _tile_map(parity)
        xT16 = xTs[b]
        cols = []
        for c in range(NCLS):
            for ht in range(2):
                r = tm[(c, ht)]
                cols.append(xT16[:, r * 128 : (r + 1) * 128])
        xq16 = np.ascontiguousarray(np.concatenate(cols, axis=1))
        in_maps.append(
            {
                "xT": xT16,
                "xq": xq16,
                "wqT": wqT,
                "wkT": wkT,
                "wvT": wvT,
                "bqs": bqs,
                "bk": bk,
                "bv": bv,
                "msk": msk_even if parity == 0 else msk_odd,
            }
        )

    nc = _build_program()
    res = None
    last_err = None
    for attempt in range(3):
        try:
            res = run_bass_kernel_spmd(nc, in_maps, list(range(NCORES)))
            break
        except Exception as e:  # transient NRT device wedge; retry
            last_err = e
            import time as _time

            _time.sleep(2.0)
    if res is None:
        raise last_err
    LAST_RESULT = res

    out = np.empty((B, S, H), dtype=np.float32)
    for core in range(NCORES):
        b, parity = core // 2, core % 2
        tm = _tile_map(parity)
        o = res.results[core]["out"]  # [8, 128, H]
        for c in range(NCLS):
            for ht in range(2):
                r = tm[(c, ht)]
                out[b, r * 128 : (r + 1) * 128, :] = o[2 * c + ht]
    return out


# revision 7
# speedup vs baseline: 1.1083x; 1.1083x over previous
import os
import sys

sys.path.insert(0, "/opt/trn_rl_repo")

import numpy as np
import ml_dtypes
import bass_rust
from concourse import bass, mybir
from concourse.tile import TileContext
from concourse.vector_clock import ScopedClock
from concourse.bass_utils import run_bass_kernel_spmd

B, S, E, H = 4, 2048, 1024, 1024
NCORES = 8
EC = E // 128  # contraction chunks
HC = H // 128  # h chunks
KC = S // 128  # key chunks (16)
NCLS = 4  # tile classes per core; each class owns 2 query tiles
F32 = mybir.dt.float32
BF = mybir.dt.bfloat16
BFNP = ml_dtypes.bfloat16

# Results of the last run_bass_kernel_spmd call (for test harness inspection).
LAST_RESULT = None


def _tile_map(parity):
    """(cls, half) -> global 128-row query tile index.

    Even cores take tiles {4c, 4c+3}, odd cores {4c+1, 4c+2}: both cores then
    process an identical padded chunk schedule (4c+2 both-tile chunks plus 2
    B-only chunks per class), with per-core mask data absorbing the
    difference.
    """
    m = {}
    for c in range(NCLS):
        if parity == 0:
            m[(c, 0)], m[(c, 1)] = 4 * c, 4 * c + 3
        else:
            m[(c, 0)], m[(c, 1)] = 4 * c + 1, 4 * c + 2
    return m


class PatchedTileContext(TileContext):
    """TileContext whose tail drain carries at most one sem wait.

    The walrus codegen in this container rejects a Drain with more than one
    sync wait ("Too many sync wait commands"); split the global-clock waits
    across a chain of drains on the same engine instead.
    """

    def _drain_and_barrier(self, tick_clock, wait_clock):
        drain_inst = self.nc.sync.drain()
        wait_clock.add_sem_waits(
            drain_inst.ins, ScopedClock({None: tick_clock.global_clock})
        )
        mi = drain_inst.ins
        waits = list(mi.sync_info.on_wait)
        ups = list(mi.sync_info.on_update)
        if len(waits) > 1:
            mi.sync_info = bass_rust.SyncInfo(on_wait=waits[:1], on_update=[])
            for i, w in enumerate(waits[1:]):
                last = i == len(waits) - 2
                d2 = self.nc.sync.drain()
                d2.ins.sync_info = bass_rust.SyncInfo(
                    on_wait=[w], on_update=ups if last else []
                )
        self.nc.all_engine_barrier()
        assert self.sems is not None
        popped = self.nc._tile_sem_poison_stack.pop()
        assert popped is self._sem_poison
        self.nc.clear_and_free_semaphores(list(self.sems.allocated().values()))
        self.nc.all_engine_barrier()


def _split_multi_waits(json_bytes):
    """Rewrite BIR so no instruction carries more than one sync wait."""
    import json as _json

    d = _json.loads(json_bytes)
    ctr = 0
    for f in d.get("functions", []):
        for blk in f.get("blocks", []):
            insts = blk.get("instructions", [])
            out = []
            for inst in insts:
                si = inst.get("sync_info") or {}
                ow = si.get("on_wait") or []
                if len(ow) > 1:
                    for w in ow[:-1]:
                        out.append(
                            {
                                "debug": inst.get("debug", 0),
                                "engine": inst["engine"],
                                "ins": [],
                                "name": f"wsplit_{ctr}",
                                "opcode": "NoOp",
                                "outs": [],
                                "sync_info": {"on_update": [], "on_wait": [w]},
                            }
                        )
                        ctr += 1
                    si = dict(si)
                    si["on_wait"] = [ow[-1]]
                    inst = dict(inst)
                    inst["sync_info"] = si
                out.append(inst)
            blk["instructions"] = out
    return _json.dumps(d).encode()


def _build_program():
    nc = bass.Bass("TRN2", target_bir_lowering=False, debug=False, num_devices=NCORES)
    orig_to_json_bytes = nc.to_json_bytes
    nc.to_json_bytes = lambda: _split_multi_waits(orig_to_json_bytes())

    xT = nc.dram_tensor("xT", [E, S], BF, kind="ExternalInput")
    xq = nc.dram_tensor("xq", [E, NCLS * 256], BF, kind="ExternalInput")
    wqT = nc.dram_tensor("wqT", [E, H], BF, kind="ExternalInput")
    wkT = nc.dram_tensor("wkT", [E, H], BF, kind="ExternalInput")
    wvT = nc.dram_tensor("wvT", [E, H], BF, kind="ExternalInput")
    bqs = nc.dram_tensor("bqs", [H], F32, kind="ExternalInput")
    bk = nc.dram_tensor("bk", [H], F32, kind="ExternalInput")
    bv = nc.dram_tensor("bv", [H], F32, kind="ExternalInput")
    msk = nc.dram_tensor("msk", [4, 128, 128], BF, kind="ExternalInput")
    out = nc.dram_tensor("out", [2 * NCLS, 128, H], F32, kind="ExternalOutput")

    with PatchedTileContext(nc) as tc:
        with (
            tc.tile_pool(name="const", bufs=1) as const_pool,
            tc.tile_pool(name="xp", bufs=1) as x_pool,
            tc.tile_pool(name="ktp", bufs=1) as kt_pool,
            tc.tile_pool(name="qtp", bufs=1) as qt_pool,
            tc.tile_pool(name="vp", bufs=1) as v_pool,
        ):
            bcst = const_pool.tile([128, 2 * HC], F32, tag="bcst")
            bq_t = bcst[:, 0:HC]
            bk_t = bcst[:, HC : 2 * HC]
            bvb = const_pool.tile([128, H], F32, tag="bvb")
            msk_sb = const_pool.tile([128, 4, 128], BF, tag="msk")
            ones_t = const_pool.tile([128, 64], BF, tag="ones")
            nc.vector.memset(ones_t, 1.0)
            nc.gpsimd.dma_start(out=bq_t, in_=bqs[:].rearrange("(c p) -> p c", p=128))
            nc.gpsimd.dma_start(out=bk_t, in_=bk[:].rearrange("(c p) -> p c", p=128))

            x_sb = x_pool.tile([128, EC, S], BF, tag="x")
            kt = kt_pool.tile([128, HC, S], BF, tag="kt")
            qt = qt_pool.tile([128, HC, NCLS * 256], BF, tag="qt")
            v_sb = v_pool.tile([128, KC, H], BF, tag="v")

            with (
                tc.tile_pool(name="wkp", bufs=1) as wk_pool,
                tc.tile_pool(name="wqp", bufs=1) as wq_pool,
                tc.tile_pool(name="wvp", bufs=1) as wv_pool,
                tc.tile_pool(name="xqp", bufs=1) as xq_pool,
            ):
                wk_sb = wk_pool.tile([128, EC, H], BF, tag="wk")
                wq_sb = wq_pool.tile([128, EC, H], BF, tag="wq")
                wv_sb = wv_pool.tile([128, EC, H], BF, tag="wv")
                xq_sb = xq_pool.tile([128, EC, NCLS * 256], BF, tag="xq")

                # critical-path stream on sync, strict priority order: the
                # first K-proj slice only needs x cols 0:512, so ship those
                # first and backfill the rest while slice 0 computes.
                for e in range(EC):
                    nc.sync.dma_start(
                        out=wk_sb[:, e, :], in_=wkT[e * 128 : (e + 1) * 128, :]
                    )
                    nc.sync.dma_start(
                        out=x_sb[:, e, 0:512], in_=xT[e * 128 : (e + 1) * 128, 0:512]
                    )
                for e in range(EC):
                    nc.sync.dma_start(
                        out=x_sb[:, e, 512:S], in_=xT[e * 128 : (e + 1) * 128, 512:S]
                    )
                for e in range(EC):
                    nc.sync.dma_start(
                        out=xq_sb[:, e, :], in_=xq[e * 128 : (e + 1) * 128, :]
                    )
                for e in range(EC):
                    nc.sync.dma_start(
                        out=wq_sb[:, e, :], in_=wqT[e * 128 : (e + 1) * 128, :]
                    )
                for e in range(EC):
                    nc.sync.dma_start(
                        out=wv_sb[:, e, :], in_=wvT[e * 128 : (e + 1) * 128, :]
                    )
                nc.sync.dma_start(out=bvb, in_=bv[:].partition_broadcast(128))
                nc.sync.dma_start(
                    out=msk_sb, in_=msk[:, :, :].rearrange("m p q -> p m q")
                )

                # ---- K projection: kt[h, s] = (x @ Wk.T).T in bf16 ---------
                with tc.tile_pool(name="pskp", bufs=1, space="PSUM") as psk_pool:
                    for sl in range(4):
                        psk = [
                            psk_pool.tile([128, 512], F32, tag=f"psk{hc}", name=f"psk{hc}")
                            for hc in range(HC)
                        ]
                        for e in range(EC):
                            for hc in range(HC):
                                nc.tensor.matmul(
                                    psk[hc],
                                    lhsT=wk_sb[:, e, hc * 128 : (hc + 1) * 128],
                                    rhs=x_sb[:, e, sl * 512 : (sl + 1) * 512],
                                    start=(e == 0),
                                    stop=(e == EC - 1),
                                )
                        for hc in range(HC):
                            nc.vector.tensor_scalar_add(
                                kt[:, hc, sl * 512 : (sl + 1) * 512],
                                psk[hc],
                                bk_t[:, hc : hc + 1],
                            )

                # ---- Q projection (own tiles only) -------------------------
                with tc.tile_pool(name="psqp", bufs=1, space="PSUM") as psq_pool:
                    for c in range(NCLS):
                        psq = [
                            psq_pool.tile([128, 256], F32, tag=f"psq{hc}", name=f"psq{hc}")
                            for hc in range(HC)
                        ]
                        for e in range(EC):
                            for hc in range(HC):
                                nc.tensor.matmul(
                                    psq[hc],
                                    lhsT=wq_sb[:, e, hc * 128 : (hc + 1) * 128],
                                    rhs=xq_sb[:, e, c * 256 : (c + 1) * 256],
                                    start=(e == 0),
                                    stop=(e == EC - 1),
                                )
                        for hc in range(HC):
                            nc.vector.tensor_scalar_add(
                                qt[:, hc, c * 256 : (c + 1) * 256],
                                psq[hc],
                                bq_t[:, hc : hc + 1],
                            )

                # ---- V projection: v[s, h] per 128-row key chunk -----------
                with tc.tile_pool(name="psvp", bufs=2, space="PSUM") as psv_pool:
                    for kc in range(KC):
                        psv = [
                            psv_pool.tile([128, 512], F32, tag=f"psv{hh}", name=f"psv{hh}")
                            for hh in range(2)
                        ]
                        for e in range(EC):
                            for hh in range(2):
                                nc.tensor.matmul(
                                    psv[hh],
                                    lhsT=x_sb[:, e, kc * 128 : (kc + 1) * 128],
                                    rhs=wv_sb[:, e, hh * 512 : (hh + 1) * 512],
                                    start=(e == 0),
                                    stop=(e == EC - 1),
                                )
                        for hh in range(2):
                            nc.vector.tensor_add(
                                v_sb[:, kc, hh * 512 : (hh + 1) * 512],
                                psv[hh],
                                bvb[:, hh * 512 : (hh + 1) * 512],
                            )

            # ---- attention ------------------------------------------------
            # S^T layout: scores land [key, query] in PSUM, exp writes P^T
            # straight to SBUF (no transposes, no max subtraction needed),
            # and the softmax denominator comes from a width-1 ones matmul
            # accumulated alongside the AV matmuls.
            with (
                tc.tile_pool(name="ptp", bufs=2) as pt_pool,
                tc.tile_pool(name="outp", bufs=2) as out_pool,
                tc.tile_pool(name="stat", bufs=4) as stat_pool,
                tc.tile_pool(name="spp", bufs=2, space="PSUM") as sp_pool,
                tc.tile_pool(name="pop", bufs=1, space="PSUM") as po_pool,
                tc.tile_pool(name="lp", bufs=1, space="PSUM") as l_pool,
            ):
                for c in reversed(range(NCLS)):
                    nboth, ntot = 4 * c + 2, 4 * c + 4
                    pt = pt_pool.tile([128, KC, 256], BF, tag="pt", name=f"pt{c}")
                    po = [
                        [
                            po_pool.tile(
                                [128, 512], F32, tag=f"po{ht}{hh}", name=f"po{c}_{ht}{hh}"
                            )
                            for hh in range(2)
                        ]
                        for ht in range(2)
                    ]
                    lps = [
                        l_pool.tile([128, 64], F32, tag=f"l{ht}", name=f"l{c}_{ht}")
                        for ht in range(2)
                    ]

                    def scores(kc):
                        wide = kc < nboth
                        sp = sp_pool.tile([128, 256], F32, tag="sp", name=f"sp{c}_{kc}")
                        spo = sp if wide else sp[:, 0:128]
                        q0 = c * 256 if wide else c * 256 + 128
                        qw = 256 if wide else 128
                        for hc in range(HC):
                            nc.tensor.matmul(
                                spo,
                                lhsT=kt[:, hc, kc * 128 : (kc + 1) * 128],
                                rhs=qt[:, hc, q0 : q0 + qw],
                                start=(hc == 0),
                                stop=(hc == HC - 1),
                            )
                        dst = pt[:, kc, 0:256] if wide else pt[:, kc, 128:256]
                        nc.scalar.activation(
                            dst, spo, mybir.ActivationFunctionType.Exp
                        )
                        pos = kc - 4 * c
                        if 0 <= pos < 4:
                            ht = 0 if pos < 2 else 1
                            sl = pt[:, kc, ht * 128 : (ht + 1) * 128]
                            nc.vector.tensor_mul(sl, sl, msk_sb[:, pos, :])

                    def av(kc):
                        for ht in range(2):
                            last = nboth - 1 if ht == 0 else ntot - 1
                            if kc > last:
                                continue
                            lhsT = pt[:, kc, ht * 128 : (ht + 1) * 128]
                            for hh in range(2):
                                nc.tensor.matmul(
                                    po[ht][hh],
                                    lhsT=lhsT,
                                    rhs=v_sb[:, kc, hh * 512 : (hh + 1) * 512],
                                    start=(kc == 0),
                                    stop=(kc == last),
                                )
                            nc.tensor.matmul(
                                lps[ht],
                                lhsT=lhsT,
                                rhs=ones_t,
                                start=(kc == 0),
                                stop=(kc == last),
                            )

                    for kc in range(ntot):
                        scores(kc)
                        if kc >= 2:
                            av(kc - 2)
                    av(ntot - 2)
                    av(ntot - 1)

                    for ht in range(2):
                        rl = stat_pool.tile([128, 1], F32, tag="rl", name=f"rl{c}_{ht}")
                        nc.vector.reciprocal(rl, lps[ht][:, 0:1])
                        ot = out_pool.tile([128, H], F32, tag="ot", name=f"ot{c}_{ht}")
                        for hh in range(2):
                            nc.vector.tensor_scalar_mul(
                                ot[:, hh * 512 : (hh + 1) * 512], po[ht][hh], rl
                            )
                        nc.sync.dma_start(out=out[2 * c + ht, :, :], in_=ot)

    return nc


def kernel(inputs, Wq, bq, Wk, bk, Wv, bv):
    global LAST_RESULT
    inputs = np.ascontiguousarray(inputs, dtype=np.float32)
    scale = np.float32(1.0 / np.sqrt(np.float32(E)))

    wqT = np.ascontiguousarray((Wq.T.astype(np.float32) * scale)).astype(BFNP)
    wkT = np.ascontiguousarray(Wk.T.astype(np.float32)).astype(BFNP)
    wvT = np.ascontiguousarray(Wv.T.astype(np.float32)).astype(BFNP)
    bqs = (bq.astype(np.float32) * scale).copy()
    bk = np.ascontiguousarray(bk, dtype=np.float32)
    bv = np.ascontiguousarray(bv, dtype=np.float32)

    # mask pages: [A@4c, A@4c+1, B@4c+2, B@4c+3]; tri[k, q] = 1 iff k <= q
    kk = np.arange(128)[:, None]
    qq = np.arange(128)[None, :]
    tri = (kk <= qq).astype(np.float32)
    onesm = np.ones((128, 128), np.float32)
    zerom = np.zeros((128, 128), np.float32)
    msk_even = np.stack([tri, zerom, onesm, tri]).astype(BFNP)
    msk_odd = np.stack([onesm, tri, tri, zerom]).astype(BFNP)

    xTs = [np.ascontiguousarray(inputs[b].T).astype(BFNP) for b in range(B)]

    in_maps = []
    for core in range(NCORES):
        b, parity = core // 2, core % 2
        tm = _tile_map(parity)
        xT16 = xTs[b]
        cols = []
        for c in range(NCLS):
            for ht in range(2):
                r = tm[(c, ht)]
                cols.append(xT16[:, r * 128 : (r + 1) * 128])
        xq16 = np.ascontiguousarray(np.concatenate(cols, axis=1))
        in_maps.append(
            {
                "xT": xT16,
                "xq": xq16,
                "wqT": wqT,
                "wkT": wkT,
                "wvT": wvT,
                "bqs": bqs,
                "bk": bk,
                "bv": bv,
                "msk": msk_even if parity == 0 else msk_odd,
            }
        )

    nc = _build_program()
    res = None
    last_err = None
    for attempt in range(3):
        try:
            res = run_bass_kernel_spmd(nc, in_maps, list(range(NCORES)))
            break
        except Exception as e:  # transient NRT device wedge; retry
            last_err = e
            import time as _time

            _time.sleep(2.0)
    if res is None:
        raise last_err
    LAST_RESULT = res

    out = np.empty((B, S, H), dtype=np.float32)
    for core in range(NCORES):
        b, parity = core // 2, core % 2
        tm = _tile_map(parity)
        o = res.results[core]["out"]  # [8, 128, H]
        for c in range(NCLS):
            for ht in range(2):
                r = tm[(c, ht)]
                out[b, r * 128 : (r + 1) * 128, :] = o[2 * c + ht]
    return out


# revision 10
# speedup vs baseline: 1.1117x; 1.0031x over previous
import os
import sys

sys.path.insert(0, "/opt/trn_rl_repo")

import numpy as np
import ml_dtypes
import bass_rust
from concourse import bass, mybir
from concourse.tile import TileContext
from concourse.vector_clock import ScopedClock
from concourse.bass_utils import run_bass_kernel_spmd

B, S, E, H = 4, 2048, 1024, 1024
NCORES = 8
EC = E // 128  # contraction chunks
HC = H // 128  # h chunks
KC = S // 128  # key chunks (16)
NCLS = 4  # tile classes per core; each class owns 2 query tiles
F32 = mybir.dt.float32
BF = mybir.dt.bfloat16
BFNP = ml_dtypes.bfloat16

# Results of the last run_bass_kernel_spmd call (for test harness inspection).
LAST_RESULT = None


def _tile_map(parity):
    """(cls, half) -> global 128-row query tile index.

    Even cores take tiles {4c, 4c+3}, odd cores {4c+1, 4c+2}: both cores then
    process an identical padded chunk schedule (4c+2 both-tile chunks plus 2
    B-only chunks per class), with per-core mask data absorbing the
    difference.
    """
    m = {}
    for c in range(NCLS):
        if parity == 0:
            m[(c, 0)], m[(c, 1)] = 4 * c, 4 * c + 3
        else:
            m[(c, 0)], m[(c, 1)] = 4 * c + 1, 4 * c + 2
    return m


class PatchedTileContext(TileContext):
    """TileContext whose tail drain carries at most one sem wait.

    The walrus codegen in this container rejects a Drain with more than one
    sync wait ("Too many sync wait commands"); split the global-clock waits
    across a chain of drains on the same engine instead.
    """

    def _drain_and_barrier(self, tick_clock, wait_clock):
        drain_inst = self.nc.sync.drain()
        wait_clock.add_sem_waits(
            drain_inst.ins, ScopedClock({None: tick_clock.global_clock})
        )
        mi = drain_inst.ins
        waits = list(mi.sync_info.on_wait)
        ups = list(mi.sync_info.on_update)
        if len(waits) > 1:
            mi.sync_info = bass_rust.SyncInfo(on_wait=waits[:1], on_update=[])
            for i, w in enumerate(waits[1:]):
                last = i == len(waits) - 2
                d2 = self.nc.sync.drain()
                d2.ins.sync_info = bass_rust.SyncInfo(
                    on_wait=[w], on_update=ups if last else []
                )
        self.nc.all_engine_barrier()
        assert self.sems is not None
        popped = self.nc._tile_sem_poison_stack.pop()
        assert popped is self._sem_poison
        self.nc.clear_and_free_semaphores(list(self.sems.allocated().values()))
        self.nc.all_engine_barrier()


def _split_multi_waits(json_bytes):
    """Rewrite BIR so no instruction carries more than one sync wait."""
    import json as _json

    d = _json.loads(json_bytes)
    ctr = 0
    for f in d.get("functions", []):
        for blk in f.get("blocks", []):
            insts = blk.get("instructions", [])
            out = []
            for inst in insts:
                si = inst.get("sync_info") or {}
                ow = si.get("on_wait") or []
                if len(ow) > 1:
                    for w in ow[:-1]:
                        out.append(
                            {
                                "debug": inst.get("debug", 0),
                                "engine": inst["engine"],
                                "ins": [],
                                "name": f"wsplit_{ctr}",
                                "opcode": "NoOp",
                                "outs": [],
                                "sync_info": {"on_update": [], "on_wait": [w]},
                            }
                        )
                        ctr += 1
                    si = dict(si)
                    si["on_wait"] = [ow[-1]]
                    inst = dict(inst)
                    inst["sync_info"] = si
                out.append(inst)
            blk["instructions"] = out
    return _json.dumps(d).encode()


def _build_program():
    nc = bass.Bass("TRN2", target_bir_lowering=False, debug=False, num_devices=NCORES)
    orig_to_json_bytes = nc.to_json_bytes
    nc.to_json_bytes = lambda: _split_multi_waits(orig_to_json_bytes())

    xT = nc.dram_tensor("xT", [E, S], BF, kind="ExternalInput")
    xq = nc.dram_tensor("xq", [E, NCLS * 256], BF, kind="ExternalInput")
    wqT = nc.dram_tensor("wqT", [E, H], BF, kind="ExternalInput")
    wkT = nc.dram_tensor("wkT", [E, H], BF, kind="ExternalInput")
    wvT = nc.dram_tensor("wvT", [E, H], BF, kind="ExternalInput")
    bqs = nc.dram_tensor("bqs", [H], F32, kind="ExternalInput")
    bk = nc.dram_tensor("bk", [H], F32, kind="ExternalInput")
    bv = nc.dram_tensor("bv", [H], F32, kind="ExternalInput")
    msk = nc.dram_tensor("msk", [4, 128, 128], BF, kind="ExternalInput")
    out = nc.dram_tensor("out", [2 * NCLS, 128, H], F32, kind="ExternalOutput")

    with PatchedTileContext(nc) as tc:
        with (
            tc.tile_pool(name="const", bufs=1) as const_pool,
            tc.tile_pool(name="xp", bufs=1) as x_pool,
            tc.tile_pool(name="ktp", bufs=1) as kt_pool,
            tc.tile_pool(name="qtp", bufs=1) as qt_pool,
            tc.tile_pool(name="vp", bufs=1) as v_pool,
        ):
            bcst = const_pool.tile([128, 2 * HC], F32, tag="bcst")
            bq_t = bcst[:, 0:HC]
            bk_t = bcst[:, HC : 2 * HC]
            bvb = const_pool.tile([128, H], F32, tag="bvb")
            msk_sb = const_pool.tile([128, 4, 128], BF, tag="msk")
            ones_t = const_pool.tile([128, 64], BF, tag="ones")
            nc.vector.memset(ones_t, 1.0)
            nc.gpsimd.dma_start(out=bq_t, in_=bqs[:].rearrange("(c p) -> p c", p=128))
            nc.gpsimd.dma_start(out=bk_t, in_=bk[:].rearrange("(c p) -> p c", p=128))

            x_sb = x_pool.tile([128, EC, S], BF, tag="x")
            kt = kt_pool.tile([128, HC, S], BF, tag="kt")
            qt = qt_pool.tile([128, HC, NCLS * 256], BF, tag="qt")
            v_sb = v_pool.tile([128, KC, H], BF, tag="v")

            with (
                tc.tile_pool(name="wkp", bufs=1) as wk_pool,
                tc.tile_pool(name="wqp", bufs=1) as wq_pool,
                tc.tile_pool(name="wvp", bufs=1) as wv_pool,
                tc.tile_pool(name="xqp", bufs=1) as xq_pool,
            ):
                wk_sb = wk_pool.tile([128, EC, H], BF, tag="wk")
                wq_sb = wq_pool.tile([128, EC, H], BF, tag="wq")
                wv_sb = wv_pool.tile([128, EC, H], BF, tag="wv")
                xq_sb = xq_pool.tile([128, EC, NCLS * 256], BF, tag="xq")

                # critical-path stream on sync, strict priority order: the
                # first K-proj slice only needs x cols 0:512, so ship those
                # first and backfill the rest while slice 0 computes.
                nc.sync.dma_start(out=wk_sb[:, 0, 0:512], in_=wkT[0:128, 0:512])
                nc.sync.dma_start(out=x_sb[:, 0, 0:512], in_=xT[0:128, 0:512])
                nc.sync.dma_start(out=wk_sb[:, 0, 512:H], in_=wkT[0:128, 512:H])
                for e in range(1, EC):
                    nc.sync.dma_start(
                        out=wk_sb[:, e, :], in_=wkT[e * 128 : (e + 1) * 128, :]
                    )
                    nc.sync.dma_start(
                        out=x_sb[:, e, 0:512], in_=xT[e * 128 : (e + 1) * 128, 0:512]
                    )
                for e in range(EC):
                    nc.sync.dma_start(
                        out=x_sb[:, e, 512:S], in_=xT[e * 128 : (e + 1) * 128, 512:S]
                    )
                for e in range(EC):
                    nc.sync.dma_start(
                        out=xq_sb[:, e, :], in_=xq[e * 128 : (e + 1) * 128, :]
                    )
                for e in range(EC):
                    nc.sync.dma_start(
                        out=wq_sb[:, e, :], in_=wqT[e * 128 : (e + 1) * 128, :]
                    )
                for e in range(EC):
                    nc.sync.dma_start(
                        out=wv_sb[:, e, :], in_=wvT[e * 128 : (e + 1) * 128, :]
                    )
                nc.sync.dma_start(out=bvb, in_=bv[:].partition_broadcast(128))
                nc.sync.dma_start(
                    out=msk_sb, in_=msk[:, :, :].rearrange("m p q -> p m q")
                )

                # ---- K projection: kt[h, s] = (x @ Wk.T).T in bf16 ---------
                with tc.tile_pool(name="pskp", bufs=1, space="PSUM") as psk_pool:
                    for sl in range(4):
                        psk = [
                            psk_pool.tile([128, 512], F32, tag=f"psk{hc}", name=f"psk{hc}")
                            for hc in range(HC)
                        ]
                        for e in range(EC):
                            for hc in range(HC):
                                nc.tensor.matmul(
                                    psk[hc],
                                    lhsT=wk_sb[:, e, hc * 128 : (hc + 1) * 128],
                                    rhs=x_sb[:, e, sl * 512 : (sl + 1) * 512],
                                    start=(e == 0),
                                    stop=(e == EC - 1),
                                )
                        for hc in range(HC):
                            dst = kt[:, hc, sl * 512 : (sl + 1) * 512]
                            if hc % 2 == 0:
                                nc.vector.tensor_scalar_add(
                                    dst, psk[hc], bk_t[:, hc : hc + 1]
                                )
                            else:
                                nc.scalar.activation(
                                    dst,
                                    psk[hc],
                                    mybir.ActivationFunctionType.Identity,
                                    bias=bk_t[:, hc : hc + 1],
                                )

                # ---- Q projection (own tiles only) -------------------------
                with tc.tile_pool(name="psqp", bufs=1, space="PSUM") as psq_pool:
                    for c in range(NCLS):
                        psq = [
                            psq_pool.tile([128, 256], F32, tag=f"psq{hc}", name=f"psq{hc}")
                            for hc in range(HC)
                        ]
                        for e in range(EC):
                            for hc in range(HC):
                                nc.tensor.matmul(
                                    psq[hc],
                                    lhsT=wq_sb[:, e, hc * 128 : (hc + 1) * 128],
                                    rhs=xq_sb[:, e, c * 256 : (c + 1) * 256],
                                    start=(e == 0),
                                    stop=(e == EC - 1),
                                )
                        for hc in range(HC):
                            dst = qt[:, hc, c * 256 : (c + 1) * 256]
                            if hc % 2 == 0:
                                nc.vector.tensor_scalar_add(
                                    dst, psq[hc], bq_t[:, hc : hc + 1]
                                )
                            else:
                                nc.scalar.activation(
                                    dst,
                                    psq[hc],
                                    mybir.ActivationFunctionType.Identity,
                                    bias=bq_t[:, hc : hc + 1],
                                )

                # ---- V projection: v[s, h] per 128-row key chunk -----------
                with tc.tile_pool(name="psvp", bufs=2, space="PSUM") as psv_pool:
                    for kc in range(KC):
                        psv = [
                            psv_pool.tile([128, 512], F32, tag=f"psv{hh}", name=f"psv{hh}")
                            for hh in range(2)
                        ]
                        for e in range(EC):
                            for hh in range(2):
                                nc.tensor.matmul(
                                    psv[hh],
                                    lhsT=x_sb[:, e, kc * 128 : (kc + 1) * 128],
                                    rhs=wv_sb[:, e, hh * 512 : (hh + 1) * 512],
                                    start=(e == 0),
                                    stop=(e == EC - 1),
                                )
                        nc.vector.tensor_add(
                            v_sb[:, kc, 0:512], psv[0], bvb[:, 0:512]
                        )
                        nc.vector.tensor_add(
                            v_sb[:, kc, 512:1024], psv[1], bvb[:, 512:1024]
                        )

            # ---- attention ------------------------------------------------
            # S^T layout: scores land [key, query] in PSUM, exp writes P^T
            # straight to SBUF (no transposes, no max subtraction needed),
            # and the softmax denominator comes from a width-1 ones matmul
            # accumulated alongside the AV matmuls.
            with (
                tc.tile_pool(name="ptp", bufs=2) as pt_pool,
                tc.tile_pool(name="outp", bufs=2) as out_pool,
                tc.tile_pool(name="stat", bufs=4) as stat_pool,
                tc.tile_pool(name="spp", bufs=2, space="PSUM") as sp_pool,
                tc.tile_pool(name="pop", bufs=1, space="PSUM") as po_pool,
                tc.tile_pool(name="lp", bufs=1, space="PSUM") as l_pool,
            ):
                for c in reversed(range(NCLS)):
                    nboth, ntot = 4 * c + 2, 4 * c + 4
                    pt = pt_pool.tile([128, KC, 256], BF, tag="pt", name=f"pt{c}")
                    po = [
                        [
                            po_pool.tile(
                                [128, 512], F32, tag=f"po{ht}{hh}", name=f"po{c}_{ht}{hh}"
                            )
                            for hh in range(2)
                        ]
                        for ht in range(2)
                    ]
                    lps = [
                        l_pool.tile([128, 64], F32, tag=f"l{ht}", name=f"l{c}_{ht}")
                        for ht in range(2)
                    ]

                    def scores(kc):
                        wide = kc < nboth
                        sp = sp_pool.tile([128, 256], F32, tag="sp", name=f"sp{c}_{kc}")
                        spo = sp if wide else sp[:, 0:128]
                        q0 = c * 256 if wide else c * 256 + 128
                        qw = 256 if wide else 128
                        for hc in range(HC):
                            nc.tensor.matmul(
                                spo,
                                lhsT=kt[:, hc, kc * 128 : (kc + 1) * 128],
                                rhs=qt[:, hc, q0 : q0 + qw],
                                start=(hc == 0),
                                stop=(hc == HC - 1),
                            )
                        dst = pt[:, kc, 0:256] if wide else pt[:, kc, 128:256]
                        nc.scalar.activation(
                            dst, spo, mybir.ActivationFunctionType.Exp
                        )
                        pos = kc - 4 * c
                        if 0 <= pos < 4:
                            ht = 0 if pos < 2 else 1
                            sl = pt[:, kc, ht * 128 : (ht + 1) * 128]
                            nc.vector.tensor_mul(sl, sl, msk_sb[:, pos, :])

                    def av(kc):
                        for ht in range(2):
                            last = nboth - 1 if ht == 0 else ntot - 1
                            if kc > last:
                                continue
                            lhsT = pt[:, kc, ht * 128 : (ht + 1) * 128]
                            for hh in range(2):
                                nc.tensor.matmul(
                                    po[ht][hh],
                                    lhsT=lhsT,
                                    rhs=v_sb[:, kc, hh * 512 : (hh + 1) * 512],
                                    start=(kc == 0),
                                    stop=(kc == last),
                                )
                            nc.tensor.matmul(
                                lps[ht],
                                lhsT=lhsT,
                                rhs=ones_t,
                                start=(kc == 0),
                                stop=(kc == last),
                            )

                    for kc in range(ntot):
                        scores(kc)
                        if kc >= 2:
                            av(kc - 2)
                    av(ntot - 2)
                    av(ntot - 1)

                    for ht in range(2):
                        rl = stat_pool.tile([128, 1], F32, tag="rl", name=f"rl{c}_{ht}")
                        nc.vector.reciprocal(rl, lps[ht][:, 0:1])
                        ot = out_pool.tile([128, H], F32, tag="ot", name=f"ot{c}_{ht}")
                        for hh in range(2):
                            dst = ot[:, hh * 512 : (hh + 1) * 512]
                            if ht == 0:
                                nc.vector.tensor_scalar_mul(dst, po[ht][hh], rl)
                            else:
                                nc.scalar.activation(
                                    dst,
                                    po[ht][hh],
                                    mybir.ActivationFunctionType.Copy,
                                    scale=rl,
                                )
                            nc.sync.dma_start(
                                out=out[2 * c + ht, :, hh * 512 : (hh + 1) * 512],
                                in_=dst,
                            )

    return nc


def kernel(inputs, Wq, bq, Wk, bk, Wv, bv):
    global LAST_RESULT
    inputs = np.ascontiguousarray(inputs, dtype=np.float32)
    scale = np.float32(1.0 / np.sqrt(np.float32(E)))

    wqT = np.ascontiguousarray((Wq.T.astype(np.float32) * scale)).astype(BFNP)
    wkT = np.ascontiguousarray(Wk.T.astype(np.float32)).astype(BFNP)
    wvT = np.ascontiguousarray(Wv.T.astype(np.float32)).astype(BFNP)
    bqs = (bq.astype(np.float32) * scale).copy()
    bk = np.ascontiguousarray(bk, dtype=np.float32)
    bv = np.ascontiguousarray(bv, dtype=np.float32)

    # mask pages: [A@4c, A@4c+1, B@4c+2, B@4c+3]; tri[k, q] = 1 iff k <= q
    kk = np.arange(128)[:, None]
    qq = np.arange(128)[None, :]
    tri = (kk <= qq).astype(np.float32)
    onesm = np.ones((128, 128), np.float32)
    zerom = np.zeros((128, 128), np.float32)
    msk_even = np.stack([tri, zerom, onesm, tri]).astype(BFNP)
    msk_odd = np.stack([onesm, tri, tri, zerom]).astype(BFNP)

    xTs = [np.ascontiguousarray(inputs[b].T).astype(BFNP) for b in range(B)]

    in_maps = []
    for core in range(NCORES):
        b, parity = core // 2, core % 2
        tm = _tile_map(parity)
        xT16 = xTs[b]
        cols = []
        for c in range(NCLS):
            for ht in range(2):
                r = tm[(c, ht)]
                cols.append(xT16[:, r * 128 : (r + 1) * 128])
        xq16 = np.ascontiguousarray(np.concatenate(cols, axis=1))
        in_maps.append(
            {
                "xT": xT16,
                "xq": xq16,
                "wqT": wqT,
                "wkT": wkT,
                "wvT": wvT,
                "bqs": bqs,
                "bk": bk,
                "bv": bv,
                "msk": msk_even if parity == 0 else msk_odd,
            }
        )

    nc = _build_program()
    res = None
    last_err = None
    for attempt in range(3):
        try:
            res = run_bass_kernel_spmd(nc, in_maps, list(range(NCORES)))
            break
        except Exception as e:  # transient NRT device wedge; retry
            last_err = e
            import time as _time

            _time.sleep(2.0)
    if res is None:
        raise last_err
    LAST_RESULT = res

    out = np.empty((B, S, H), dtype=np.float32)
    for core in range(NCORES):
        b, parity = core // 2, core % 2
        tm = _tile_map(parity)
        o = res.results[core]["out"]  # [8, 128, H]
        for c in range(NCLS):
            for ht in range(2):
                r = tm[(c, ht)]
                out[b, r * 128 : (r + 1) * 128, :] = o[2 * c + ht]
    return out


# revision 12
# speedup vs baseline: 1.4875x; 1.3381x over previous
import os
import sys

sys.path.insert(0, "/opt/trn_rl_repo")

import numpy as np
import ml_dtypes
import bass_rust
from concourse import bass, mybir
from concourse.tile import TileContext
from concourse.vector_clock import ScopedClock
from concourse.bass_utils import run_bass_kernel_spmd

B, S, E, H = 4, 2048, 1024, 1024
NCORES = 8
EC = E // 128  # contraction chunks
KC = S // 128  # key chunks (16)
NCLS = 4  # tile classes per core; each class owns 2 query tiles
F32 = mybir.dt.float32
BF = mybir.dt.bfloat16
BFNP = ml_dtypes.bfloat16

# Results of the last run_bass_kernel_spmd call (for test harness inspection).
LAST_RESULT = None


def _tile_map(parity):
    """(cls, half) -> global 128-row query tile index.

    Even cores take tiles {4c, 4c+3}, odd cores {4c+1, 4c+2}: both cores then
    process an identical padded chunk schedule (4c+2 both-tile chunks plus 2
    B-only chunks per class), with per-core mask data absorbing the
    difference.
    """
    m = {}
    for c in range(NCLS):
        if parity == 0:
            m[(c, 0)], m[(c, 1)] = 4 * c, 4 * c + 3
        else:
            m[(c, 0)], m[(c, 1)] = 4 * c + 1, 4 * c + 2
    return m


class PatchedTileContext(TileContext):
    """TileContext whose tail drain carries at most one sem wait.

    The walrus codegen in this container rejects a Drain with more than one
    sync wait ("Too many sync wait commands"); split the global-clock waits
    across a chain of drains on the same engine instead.
    """

    def _drain_and_barrier(self, tick_clock, wait_clock):
        drain_inst = self.nc.sync.drain()
        wait_clock.add_sem_waits(
            drain_inst.ins, ScopedClock({None: tick_clock.global_clock})
        )
        mi = drain_inst.ins
        waits = list(mi.sync_info.on_wait)
        ups = list(mi.sync_info.on_update)
        if len(waits) > 1:
            mi.sync_info = bass_rust.SyncInfo(on_wait=waits[:1], on_update=[])
            for i, w in enumerate(waits[1:]):
                last = i == len(waits) - 2
                d2 = self.nc.sync.drain()
                d2.ins.sync_info = bass_rust.SyncInfo(
                    on_wait=[w], on_update=ups if last else []
                )
        self.nc.all_engine_barrier()
        assert self.sems is not None
        popped = self.nc._tile_sem_poison_stack.pop()
        assert popped is self._sem_poison
        self.nc.clear_and_free_semaphores(list(self.sems.allocated().values()))
        self.nc.all_engine_barrier()


def _split_multi_waits(json_bytes):
    """Rewrite BIR so no instruction carries more than one sync wait."""
    import json as _json

    d = _json.loads(json_bytes)
    ctr = 0
    for f in d.get("functions", []):
        for blk in f.get("blocks", []):
            insts = blk.get("instructions", [])
            out = []
            for inst in insts:
                si = inst.get("sync_info") or {}
                ow = si.get("on_wait") or []
                if len(ow) > 1:
                    for w in ow[:-1]:
                        out.append(
                            {
                                "debug": inst.get("debug", 0),
                                "engine": inst["engine"],
                                "ins": [],
                                "name": f"wsplit_{ctr}",
                                "opcode": "NoOp",
                                "outs": [],
                                "sync_info": {"on_update": [], "on_wait": [w]},
                            }
                        )
                        ctr += 1
                    si = dict(si)
                    si["on_wait"] = [ow[-1]]
                    inst = dict(inst)
                    inst["sync_info"] = si
                out.append(inst)
            blk["instructions"] = out
    return _json.dumps(d).encode()


def _build_program():
    nc = bass.Bass("TRN2", target_bir_lowering=False, debug=False, num_devices=NCORES)
    orig_to_json_bytes = nc.to_json_bytes
    nc.to_json_bytes = lambda: _split_multi_waits(orig_to_json_bytes())

    xT = nc.dram_tensor("xT", [E, S], BF, kind="ExternalInput")
    xq = nc.dram_tensor("xq", [E, NCLS * 256], BF, kind="ExternalInput")
    mT = nc.dram_tensor("mT", [E, E], BF, kind="ExternalInput")
    ct = nc.dram_tensor("ct", [E], F32, kind="ExternalInput")
    wvT = nc.dram_tensor("wvT", [E, H], BF, kind="ExternalInput")
    bv = nc.dram_tensor("bv", [H], F32, kind="ExternalInput")
    msk = nc.dram_tensor("msk", [4, 128, 128], BF, kind="ExternalInput")
    out = nc.dram_tensor("out", [2 * NCLS, 128, H], F32, kind="ExternalOutput")

    with PatchedTileContext(nc) as tc:
        with (
            tc.tile_pool(name="const", bufs=1) as const_pool,
            tc.tile_pool(name="xp", bufs=1) as x_pool,
            tc.tile_pool(name="tqp", bufs=1) as tq_pool,
            tc.tile_pool(name="vp", bufs=1) as v_pool,
        ):
            c_t = const_pool.tile([128, EC], F32, tag="ct")
            bvb = const_pool.tile([128, H], F32, tag="bvb")
            msk_sb = const_pool.tile([128, 4, 128], BF, tag="msk")
            ones_t = const_pool.tile([128, 64], BF, tag="ones")
            nc.vector.memset(ones_t, 1.0)
            nc.gpsimd.dma_start(out=c_t, in_=ct[:].rearrange("(c p) -> p c", p=128))

            # x^T stays resident through attention: it is both the V-proj
            # stationary and the scores stationary (s = t . x via M-trick).
            x_sb = x_pool.tile([128, EC, S], BF, tag="x")
            tq = tq_pool.tile([128, EC, NCLS * 256], BF, tag="tq")
            v_sb = v_pool.tile([128, KC, H], BF, tag="v")

            with (
                tc.tile_pool(name="mp", bufs=1) as m_pool,
                tc.tile_pool(name="wvp", bufs=1) as wv_pool,
                tc.tile_pool(name="xqp", bufs=1) as xq_pool,
            ):
                m_sb = m_pool.tile([128, EC, E], BF, tag="m")
                wv_sb = wv_pool.tile([128, EC, H], BF, tag="wv")
                xq_sb = xq_pool.tile([128, EC, NCLS * 256], BF, tag="xq")

                # single priority-ordered DMA stream on sync: t'-proj inputs
                # first (M chunk + xq chunk pairs), then x, wv, consts.
                nc.sync.dma_start(out=m_sb[:, 0, 0:512], in_=mT[0:128, 0:512])
                nc.sync.dma_start(out=xq_sb[:, 0, :], in_=xq[0:128, :])
                nc.sync.dma_start(out=m_sb[:, 0, 512:E], in_=mT[0:128, 512:E])
                for e in range(1, EC):
                    nc.sync.dma_start(
                        out=m_sb[:, e, :], in_=mT[e * 128 : (e + 1) * 128, :]
                    )
                    nc.sync.dma_start(
                        out=xq_sb[:, e, :], in_=xq[e * 128 : (e + 1) * 128, :]
                    )
                for e in range(EC):
                    nc.sync.dma_start(
                        out=x_sb[:, e, :], in_=xT[e * 128 : (e + 1) * 128, :]
                    )
                for e in range(EC):
                    nc.sync.dma_start(
                        out=wv_sb[:, e, :], in_=wvT[e * 128 : (e + 1) * 128, :]
                    )
                nc.sync.dma_start(out=bvb, in_=bv[:].partition_broadcast(128))
                nc.sync.dma_start(
                    out=msk_sb, in_=msk[:, :, :].rearrange("m p q -> p m q")
                )

                # ---- t' projection: tq[e', q] = (x @ M + c).T in bf16 ------
                # two passes over e, each covering a contiguous 512-wide
                # class pair of q columns (8 psum banks per pass)
                with tc.tile_pool(name="psqp", bufs=1, space="PSUM") as psq_pool:
                    for pp in range(2):
                        q0 = 512 if pp == 0 else 0  # classes {2,3} then {0,1}
                        psq = [
                            psq_pool.tile(
                                [128, 512], F32, tag=f"psq{ep}", name=f"psq{pp}_{ep}"
                            )
                            for ep in range(EC)
                        ]
                        for e in range(EC):
                            for ep in range(EC):
                                nc.tensor.matmul(
                                    psq[ep],
                                    lhsT=m_sb[:, e, ep * 128 : (ep + 1) * 128],
                                    rhs=xq_sb[:, e, q0 : q0 + 512],
                                    start=(e == 0),
                                    stop=(e == EC - 1),
                                )
                        for ep in range(EC):
                            dst = tq[:, ep, q0 : q0 + 512]
                            if ep % 2 == 0:
                                nc.vector.tensor_scalar_add(
                                    dst, psq[ep], c_t[:, ep : ep + 1]
                                )
                            else:
                                nc.scalar.activation(
                                    dst,
                                    psq[ep],
                                    mybir.ActivationFunctionType.Identity,
                                    bias=c_t[:, ep : ep + 1],
                                )

                # ---- V projection: v[s, h] per 128-row key chunk -----------
                with tc.tile_pool(name="psvp", bufs=2, space="PSUM") as psv_pool:
                    for kc in range(KC):
                        psv = [
                            psv_pool.tile(
                                [128, 512], F32, tag=f"psv{hh}", name=f"psv{kc}_{hh}"
                            )
                            for hh in range(2)
                        ]
                        for e in range(EC):
                            for hh in range(2):
                                nc.tensor.matmul(
                                    psv[hh],
                                    lhsT=x_sb[:, e, kc * 128 : (kc + 1) * 128],
                                    rhs=wv_sb[:, e, hh * 512 : (hh + 1) * 512],
                                    start=(e == 0),
                                    stop=(e == EC - 1),
                                )
                        nc.vector.tensor_add(
                            v_sb[:, kc, 0:512], psv[0], bvb[:, 0:512]
                        )
                        nc.vector.tensor_add(
                            v_sb[:, kc, 512:1024], psv[1], bvb[:, 512:1024]
                        )

            # ---- attention ------------------------------------------------
            # S^T layout: scores land [key, query] in PSUM via
            # s^T = x^T_chunk^T . t', exp writes P^T straight to SBUF (no
            # transposes, no max subtraction), and the softmax denominator
            # comes from a width-64 ones matmul accumulated alongside AV.
            with (
                tc.tile_pool(name="ptp", bufs=2) as pt_pool,
                tc.tile_pool(name="outp", bufs=2) as out_pool,
                tc.tile_pool(name="stat", bufs=4) as stat_pool,
                tc.tile_pool(name="spp", bufs=2, space="PSUM") as sp_pool,
                tc.tile_pool(name="pop", bufs=1, space="PSUM") as po_pool,
                tc.tile_pool(name="lp", bufs=1, space="PSUM") as l_pool,
            ):
                pending_norm = [None]

                def flush_norm():
                    if pending_norm[0] is not None:
                        pending_norm[0]()
                        pending_norm[0] = None

                for c in reversed(range(NCLS)):
                    nboth, ntot = 4 * c + 2, 4 * c + 4
                    pt = pt_pool.tile([128, KC, 256], BF, tag="pt", name=f"pt{c}")
                    po = [
                        [
                            po_pool.tile(
                                [128, 512],
                                F32,
                                tag=f"po{ht}{hh}",
                                name=f"po{c}_{ht}{hh}",
                            )
                            for hh in range(2)
                        ]
                        for ht in range(2)
                    ]
                    lps = [
                        l_pool.tile([128, 64], F32, tag=f"l{ht}", name=f"l{c}_{ht}")
                        for ht in range(2)
                    ]

                    def scores(kc, c=c, nboth=nboth, pt=pt):
                        wide = kc < nboth
                        sp = sp_pool.tile([128, 256], F32, tag="sp", name=f"sp{c}_{kc}")
                        spo = sp if wide else sp[:, 0:128]
                        q0 = c * 256 if wide else c * 256 + 128
                        qw = 256 if wide else 128
                        for ec in range(EC):
                            nc.tensor.matmul(
                                spo,
                                lhsT=x_sb[:, ec, kc * 128 : (kc + 1) * 128],
                                rhs=tq[:, ec, q0 : q0 + qw],
                                start=(ec == 0),
                                stop=(ec == EC - 1),
                            )
                        dst = pt[:, kc, 0:256] if wide else pt[:, kc, 128:256]
                        nc.scalar.activation(dst, spo, mybir.ActivationFunctionType.Exp)
                        pos = kc - 4 * c
                        if 0 <= pos < 4:
                            ht = 0 if pos < 2 else 1
                            sl = pt[:, kc, ht * 128 : (ht + 1) * 128]
                            nc.vector.tensor_mul(sl, sl, msk_sb[:, pos, :])

                    def av(kc, nboth=nboth, ntot=ntot, pt=pt, po=po, lps=lps):
                        for ht in range(2):
                            last = nboth - 1 if ht == 0 else ntot - 1
                            if kc > last:
                                continue
                            lhsT = pt[:, kc, ht * 128 : (ht + 1) * 128]
                            for hh in range(2):
                                nc.tensor.matmul(
                                    po[ht][hh],
                                    lhsT=lhsT,
                                    rhs=v_sb[:, kc, hh * 512 : (hh + 1) * 512],
                                    start=(kc == 0),
                                    stop=(kc == last),
                                )
                            nc.tensor.matmul(
                                lps[ht],
                                lhsT=lhsT,
                                rhs=ones_t,
                                start=(kc == 0),
                                stop=(kc == last),
                            )

                    for kc in range(ntot):
                        scores(kc)
                        if kc == 1:
                            # normalize the previous class only after this
                            # class's first exps are queued on the scalar
                            # engine, so its AV pipeline starts undelayed
                            flush_norm()
                        if kc >= 2:
                            av(kc - 2)
                    av(ntot - 2)
                    av(ntot - 1)

                    def normalize(c=c, po=po, lps=lps):
                        for ht in range(2):
                            rl = stat_pool.tile(
                                [128, 1], F32, tag="rl", name=f"rl{c}_{ht}"
                            )
                            nc.vector.reciprocal(rl, lps[ht][:, 0:1])
                            ot = out_pool.tile(
                                [128, H], F32, tag="ot", name=f"ot{c}_{ht}"
                            )
                            for hh in range(2):
                                dst = ot[:, hh * 512 : (hh + 1) * 512]
                                if ht == 0:
                                    nc.vector.tensor_scalar_mul(dst, po[ht][hh], rl)
                                else:
                                    nc.scalar.activation(
                                        dst,
                                        po[ht][hh],
                                        mybir.ActivationFunctionType.Copy,
                                        scale=rl,
                                    )
                                nc.sync.dma_start(
                                    out=out[2 * c + ht, :, hh * 512 : (hh + 1) * 512],
                                    in_=dst,
                                )

                    pending_norm[0] = normalize
                flush_norm()

    return nc


def kernel(inputs, Wq, bq, Wk, bk, Wv, bv):
    global LAST_RESULT
    inputs = np.ascontiguousarray(inputs, dtype=np.float32)
    scale = np.float32(1.0 / np.sqrt(np.float32(E)))

    # M-trick: scores = x @ M @ x^T with M = Wq^T Wk / sqrt(E); the q-side
    # bias term cancels in softmax, the k-side term folds into t's bias.
    M = (Wq.T.astype(np.float64) @ Wk.astype(np.float64)).astype(np.float32) * scale
    cvec = (bq.astype(np.float64) @ Wk.astype(np.float64)).astype(np.float32) * scale
    mT16 = np.ascontiguousarray(M).astype(BFNP)
    wvT = np.ascontiguousarray(Wv.T.astype(np.float32)).astype(BFNP)
    bv = np.ascontiguousarray(bv, dtype=np.float32)

    # mask pages: [A@4c, A@4c+1, B@4c+2, B@4c+3]; tri[k, q] = 1 iff k <= q
    kk = np.arange(128)[:, None]
    qq = np.arange(128)[None, :]
    tri = (kk <= qq).astype(np.float32)
    onesm = np.ones((128, 128), np.float32)
    zerom = np.zeros((128, 128), np.float32)
    msk_even = np.stack([tri, zerom, onesm, tri]).astype(BFNP)
    msk_odd = np.stack([onesm, tri, tri, zerom]).astype(BFNP)

    xTs = [np.ascontiguousarray(inputs[b].T).astype(BFNP) for b in range(B)]

    in_maps = []
    for core in range(NCORES):
        b, parity = core // 2, core % 2
        tm = _tile_map(parity)
        xT16 = xTs[b]
        cols = []
        for c in range(NCLS):
            for ht in range(2):
                r = tm[(c, ht)]
                cols.append(xT16[:, r * 128 : (r + 1) * 128])
        xq16 = np.ascontiguousarray(np.concatenate(cols, axis=1))
        in_maps.append(
            {
                "xT": xT16,
                "xq": xq16,
                "mT": mT16,
                "ct": cvec,
                "wvT": wvT,
                "bv": bv,
                "msk": msk_even if parity == 0 else msk_odd,
            }
        )

    nc = _build_program()
    res = None
    last_err = None
    for attempt in range(3):
        try:
            res = run_bass_kernel_spmd(nc, in_maps, list(range(NCORES)))
            break
        except Exception as e:  # transient NRT device wedge; retry
            last_err = e
            import time as _time

            _time.sleep(2.0)
    if res is None:
        raise last_err
    LAST_RESULT = res

    out = np.empty((B, S, H), dtype=np.float32)
    for core in range(NCORES):
        b, parity = core // 2, core % 2
        tm = _tile_map(parity)
        o = res.results[core]["out"]  # [8, 128, H]
        for c in range(NCLS):
            for ht in range(2):
                r = tm[(c, ht)]
                out[b, r * 128 : (r + 1) * 128, :] = o[2 * c + ht]
    return out


# revision 13
# speedup vs baseline: 1.7921x; 1.2047x over previous
import os
import sys

sys.path.insert(0, "/opt/trn_rl_repo")

import numpy as np
import ml_dtypes
import bass_rust
from concourse import bass, mybir
from concourse.tile import TileContext
from concourse.vector_clock import ScopedClock
from concourse.bass_utils import run_bass_kernel_spmd

B, S, E, H = 4, 2048, 1024, 1024
NCORES = 8
EC = E // 128  # contraction chunks
KC = S // 128  # key chunks (16)
NCLS = 4  # tile classes per core; each class owns 2 query tiles
F32 = mybir.dt.float32
BF = mybir.dt.bfloat16
BFNP = ml_dtypes.bfloat16

# Results of the last run_bass_kernel_spmd call (for test harness inspection).
LAST_RESULT = None


def _tile_map(parity):
    """(cls, half) -> global 128-row query tile index.

    Even cores take tiles {4c, 4c+3}, odd cores {4c+1, 4c+2}: both cores then
    process an identical padded chunk schedule (4c+2 both-tile chunks plus 2
    B-only chunks per class), with per-core mask data absorbing the
    difference.
    """
    m = {}
    for c in range(NCLS):
        if parity == 0:
            m[(c, 0)], m[(c, 1)] = 4 * c, 4 * c + 3
        else:
            m[(c, 0)], m[(c, 1)] = 4 * c + 1, 4 * c + 2
    return m


class PatchedTileContext(TileContext):
    """TileContext whose tail drain carries at most one sem wait.

    The walrus codegen in this container rejects a Drain with more than one
    sync wait ("Too many sync wait commands"); split the global-clock waits
    across a chain of drains on the same engine instead.
    """

    def _drain_and_barrier(self, tick_clock, wait_clock):
        drain_inst = self.nc.sync.drain()
        wait_clock.add_sem_waits(
            drain_inst.ins, ScopedClock({None: tick_clock.global_clock})
        )
        mi = drain_inst.ins
        waits = list(mi.sync_info.on_wait)
        ups = list(mi.sync_info.on_update)
        if len(waits) > 1:
            mi.sync_info = bass_rust.SyncInfo(on_wait=waits[:1], on_update=[])
            for i, w in enumerate(waits[1:]):
                last = i == len(waits) - 2
                d2 = self.nc.sync.drain()
                d2.ins.sync_info = bass_rust.SyncInfo(
                    on_wait=[w], on_update=ups if last else []
                )
        self.nc.all_engine_barrier()
        assert self.sems is not None
        popped = self.nc._tile_sem_poison_stack.pop()
        assert popped is self._sem_poison
        self.nc.clear_and_free_semaphores(list(self.sems.allocated().values()))
        self.nc.all_engine_barrier()


def _split_multi_waits(json_bytes):
    """Rewrite BIR so no instruction carries more than one sync wait."""
    import json as _json

    d = _json.loads(json_bytes)
    ctr = 0
    for f in d.get("functions", []):
        for blk in f.get("blocks", []):
            insts = blk.get("instructions", [])
            out = []
            for inst in insts:
                si = inst.get("sync_info") or {}
                ow = si.get("on_wait") or []
                if len(ow) > 1:
                    for w in ow[:-1]:
                        out.append(
                            {
                                "debug": inst.get("debug", 0),
                                "engine": inst["engine"],
                                "ins": [],
                                "name": f"wsplit_{ctr}",
                                "opcode": "NoOp",
                                "outs": [],
                                "sync_info": {"on_update": [], "on_wait": [w]},
                            }
                        )
                        ctr += 1
                    si = dict(si)
                    si["on_wait"] = [ow[-1]]
                    inst = dict(inst)
                    inst["sync_info"] = si
                out.append(inst)
            blk["instructions"] = out
    return _json.dumps(d).encode()


def _build_program():
    nc = bass.Bass("TRN2", target_bir_lowering=False, debug=False, num_devices=NCORES)
    orig_to_json_bytes = nc.to_json_bytes
    nc.to_json_bytes = lambda: _split_multi_waits(orig_to_json_bytes())

    xT = nc.dram_tensor("xT", [E, S], BF, kind="ExternalInput")
    xN = nc.dram_tensor("xN", [S, E], BF, kind="ExternalInput")
    xq = nc.dram_tensor("xq", [E, NCLS * 256], BF, kind="ExternalInput")
    mT = nc.dram_tensor("mT", [E, E], BF, kind="ExternalInput")
    ct = nc.dram_tensor("ct", [E], F32, kind="ExternalInput")
    wvT = nc.dram_tensor("wvT", [E, H], BF, kind="ExternalInput")
    bv = nc.dram_tensor("bv", [H], F32, kind="ExternalInput")
    msk = nc.dram_tensor("msk", [4, 128, 128], BF, kind="ExternalInput")
    out = nc.dram_tensor("out", [2 * NCLS, 128, H], F32, kind="ExternalOutput")

    with PatchedTileContext(nc) as tc:
        with (
            tc.tile_pool(name="const", bufs=1) as const_pool,
            tc.tile_pool(name="xp", bufs=1) as x_pool,
            tc.tile_pool(name="tqp", bufs=1) as tq_pool,
            tc.tile_pool(name="xnp", bufs=1) as xn_pool,
            tc.tile_pool(name="wvp", bufs=1) as wv_pool,
        ):
            c_t = const_pool.tile([128, EC], F32, tag="ct")
            bvb = const_pool.tile([128, H], F32, tag="bvb")
            msk_sb = const_pool.tile([128, 4, 128], BF, tag="msk")
            ones_t = const_pool.tile([128, 64], BF, tag="ones")
            nc.vector.memset(ones_t, 1.0)
            nc.gpsimd.dma_start(out=c_t, in_=ct[:].rearrange("(c p) -> p c", p=128))

            # x^T stays resident through attention: it is both the V-proj
            # stationary and the scores stationary (s = t . x via M-trick).
            x_sb = x_pool.tile([128, EC, S], BF, tag="x")
            tq = tq_pool.tile([128, EC, NCLS * 256], BF, tag="tq")
            xn_sb = xn_pool.tile([128, KC, E], BF, tag="xn")
            wv_sb = wv_pool.tile([128, EC, H], BF, tag="wv")

            with (
                tc.tile_pool(name="mp", bufs=1) as m_pool,
                tc.tile_pool(name="xqp", bufs=1) as xq_pool,
            ):
                m_sb = m_pool.tile([128, EC, E], BF, tag="m")
                xq_sb = xq_pool.tile([128, EC, NCLS * 256], BF, tag="xq")

                # single priority-ordered DMA stream on sync: t'-proj inputs
                # first (M chunk + xq chunk pairs), then x, wv, consts.
                nc.sync.dma_start(out=m_sb[:, 0, 0:512], in_=mT[0:128, 0:512])
                nc.sync.dma_start(out=xq_sb[:, 0, :], in_=xq[0:128, :])
                nc.sync.dma_start(out=m_sb[:, 0, 512:E], in_=mT[0:128, 512:E])
                for e in range(1, EC):
                    nc.sync.dma_start(
                        out=m_sb[:, e, :], in_=mT[e * 128 : (e + 1) * 128, :]
                    )
                    nc.sync.dma_start(
                        out=xq_sb[:, e, :], in_=xq[e * 128 : (e + 1) * 128, :]
                    )
                for e in range(EC):
                    nc.sync.dma_start(
                        out=x_sb[:, e, :], in_=xT[e * 128 : (e + 1) * 128, :]
                    )
                for e in range(EC):
                    nc.sync.dma_start(
                        out=wv_sb[:, e, :], in_=wvT[e * 128 : (e + 1) * 128, :]
                    )
                for kc in range(KC):
                    nc.sync.dma_start(
                        out=xn_sb[:, kc, :], in_=xN[kc * 128 : (kc + 1) * 128, :]
                    )
                nc.sync.dma_start(out=bvb, in_=bv[:].partition_broadcast(128))
                nc.sync.dma_start(
                    out=msk_sb, in_=msk[:, :, :].rearrange("m p q -> p m q")
                )

                # ---- t' projection: tq[e', q] = (x @ M + c).T in bf16 ------
                # two passes over e, each covering a contiguous 512-wide
                # class pair of q columns (8 psum banks per pass)
                with tc.tile_pool(name="psqp", bufs=1, space="PSUM") as psq_pool:
                    for pp in range(2):
                        q0 = 512 if pp == 0 else 0  # classes {2,3} then {0,1}
                        psq = [
                            psq_pool.tile(
                                [128, 512], F32, tag=f"psq{ep}", name=f"psq{pp}_{ep}"
                            )
                            for ep in range(EC)
                        ]
                        for e in range(EC):
                            for ep in range(EC):
                                nc.tensor.matmul(
                                    psq[ep],
                                    lhsT=m_sb[:, e, ep * 128 : (ep + 1) * 128],
                                    rhs=xq_sb[:, e, q0 : q0 + 512],
                                    start=(e == 0),
                                    stop=(e == EC - 1),
                                )
                        for ep in range(EC):
                            dst = tq[:, ep, q0 : q0 + 512]
                            if ep % 2 == 0:
                                nc.vector.tensor_scalar_add(
                                    dst, psq[ep], c_t[:, ep : ep + 1]
                                )
                            else:
                                nc.scalar.activation(
                                    dst,
                                    psq[ep],
                                    mybir.ActivationFunctionType.Identity,
                                    bias=c_t[:, ep : ep + 1],
                                )

            # ---- attention ------------------------------------------------
            # Phase A: scores for all classes, S^T layout, P^T -> SBUF bf16.
            # Phase B/C per class: u = P.x (against resident natural-layout
            # x), then out = u @ Wv^T (+ l*bv after normalize). V projection
            # is folded away entirely.
            with (
                tc.tile_pool(name="ptp", bufs=4) as pt_pool,
                tc.tile_pool(name="outp", bufs=2) as out_pool,
                tc.tile_pool(name="usb", bufs=2) as u_pool_sb,
                tc.tile_pool(name="stat", bufs=4) as stat_pool,
            ):
                pts = {}
                with tc.tile_pool(name="spp", bufs=2, space="PSUM") as sp_pool:
                    for c in reversed(range(NCLS)):
                        nboth, ntot = 4 * c + 2, 4 * c + 4
                        pt = pt_pool.tile([128, KC, 256], BF, tag="pt", name=f"pt{c}")
                        pts[c] = pt
                        for kc in range(ntot):
                            wide = kc < nboth
                            sp = sp_pool.tile(
                                [128, 256], F32, tag="sp", name=f"sp{c}_{kc}"
                            )
                            spo = sp if wide else sp[:, 0:128]
                            q0 = c * 256 if wide else c * 256 + 128
                            qw = 256 if wide else 128
                            for ec in range(EC):
                                nc.tensor.matmul(
                                    spo,
                                    lhsT=x_sb[:, ec, kc * 128 : (kc + 1) * 128],
                                    rhs=tq[:, ec, q0 : q0 + qw],
                                    start=(ec == 0),
                                    stop=(ec == EC - 1),
                                )
                            dst = pt[:, kc, 0:256] if wide else pt[:, kc, 128:256]
                            nc.scalar.activation(
                                dst, spo, mybir.ActivationFunctionType.Exp
                            )
                            pos = kc - 4 * c
                            if 0 <= pos < 4:
                                ht = 0 if pos < 2 else 1
                                sl = pt[:, kc, ht * 128 : (ht + 1) * 128]
                                nc.vector.tensor_mul(sl, sl, msk_sb[:, pos, :])

                with (
                    tc.tile_pool(name="ups", bufs=1, space="PSUM") as u_pool,
                    tc.tile_pool(name="lp", bufs=1, space="PSUM") as l_pool,
                    tc.tile_pool(name="pop", bufs=2, space="PSUM") as po_pool,
                ):
                    for c in reversed(range(NCLS)):
                        nboth, ntot = 4 * c + 2, 4 * c + 4
                        pt = pts[c]
                        u_sb = u_pool_sb.tile(
                            [128, EC, 256], BF, tag="u", name=f"u{c}"
                        )
                        lps = [
                            l_pool.tile([128, 64], F32, tag=f"l{ht}", name=f"l{c}_{ht}")
                            for ht in range(2)
                        ]
                        # u^T[e, q] accumulation in two ec-half passes; the
                        # lps ones-matmuls are issued between passes so the
                        # PSUM->SBUF staging copies drain under them
                        for half in range(2):
                            ups = [
                                u_pool.tile(
                                    [128, 256], F32, tag=f"u{ei}", name=f"u{c}_{half}_{ei}"
                                )
                                for ei in range(4)
                            ]
                            for kc in range(ntot):
                                wide = kc < nboth
                                for ei in range(4):
                                    ec = half * 4 + ei
                                    dst = ups[ei] if wide else ups[ei][:, 128:256]
                                    rhs = (
                                        pt[:, kc, 0:256]
                                        if wide
                                        else pt[:, kc, 128:256]
                                    )
                                    nc.tensor.matmul(
                                        dst,
                                        lhsT=xn_sb[:, kc, ec * 128 : (ec + 1) * 128],
                                        rhs=rhs,
                                        start=(kc == 0),
                                        stop=(kc == ntot - 1),
                                    )
                            for ht in range(2):
                                if half == 0:
                                    last = nboth - 1 if ht == 0 else ntot - 1
                                    for kc in range(last + 1):
                                        nc.tensor.matmul(
                                            lps[ht],
                                            lhsT=pt[:, kc, ht * 128 : (ht + 1) * 128],
                                            rhs=ones_t,
                                            start=(kc == 0),
                                            stop=(kc == last),
                                        )
                            for ei in range(4):
                                ec = half * 4 + ei
                                if ei % 2 == 0:
                                    nc.vector.tensor_copy(
                                        u_sb[:, ec, :], ups[ei]
                                    )
                                else:
                                    nc.scalar.copy(u_sb[:, ec, :], ups[ei])

                        # out = (u @ Wv^T) * (1/l) + bv
                        for ht in range(2):
                            rl = stat_pool.tile(
                                [128, 1], F32, tag="rl", name=f"rl{c}_{ht}"
                            )
                            nc.vector.reciprocal(rl, lps[ht][:, 0:1])
                            ot = out_pool.tile(
                                [128, H], F32, tag="ot", name=f"ot{c}_{ht}"
                            )
                            for hh in range(2):
                                po = po_pool.tile(
                                    [128, 512], F32, tag="po", name=f"po{c}_{ht}{hh}"
                                )
                                for ec in range(EC):
                                    nc.tensor.matmul(
                                        po,
                                        lhsT=u_sb[:, ec, ht * 128 : (ht + 1) * 128],
                                        rhs=wv_sb[:, ec, hh * 512 : (hh + 1) * 512],
                                        start=(ec == 0),
                                        stop=(ec == EC - 1),
                                    )
                                dst = ot[:, hh * 512 : (hh + 1) * 512]
                                if ht == 0:
                                    nc.vector.tensor_scalar_mul(dst, po, rl)
                                    nc.vector.tensor_add(
                                        dst, dst, bvb[:, hh * 512 : (hh + 1) * 512]
                                    )
                                else:
                                    nc.scalar.activation(
                                        dst,
                                        po,
                                        mybir.ActivationFunctionType.Copy,
                                        scale=rl,
                                    )
                                    nc.gpsimd.tensor_add(
                                        dst, dst, bvb[:, hh * 512 : (hh + 1) * 512]
                                    )
                                nc.sync.dma_start(
                                    out=out[2 * c + ht, :, hh * 512 : (hh + 1) * 512],
                                    in_=dst,
                                )

    return nc


def kernel(inputs, Wq, bq, Wk, bk, Wv, bv):
    global LAST_RESULT
    inputs = np.ascontiguousarray(inputs, dtype=np.float32)
    scale = np.float32(1.0 / np.sqrt(np.float32(E)))

    # M-trick: scores = x @ M @ x^T with M = Wq^T Wk / sqrt(E); the q-side
    # bias term cancels in softmax, the k-side term folds into t's bias.
    M = (Wq.T.astype(np.float64) @ Wk.astype(np.float64)).astype(np.float32) * scale
    cvec = (bq.astype(np.float64) @ Wk.astype(np.float64)).astype(np.float32) * scale
    mT16 = np.ascontiguousarray(M).astype(BFNP)
    wvT = np.ascontiguousarray(Wv.T.astype(np.float32)).astype(BFNP)
    bv = np.ascontiguousarray(bv, dtype=np.float32)

    # mask pages: [A@4c, A@4c+1, B@4c+2, B@4c+3]; tri[k, q] = 1 iff k <= q
    kk = np.arange(128)[:, None]
    qq = np.arange(128)[None, :]
    tri = (kk <= qq).astype(np.float32)
    onesm = np.ones((128, 128), np.float32)
    zerom = np.zeros((128, 128), np.float32)
    msk_even = np.stack([tri, zerom, onesm, tri]).astype(BFNP)
    msk_odd = np.stack([onesm, tri, tri, zerom]).astype(BFNP)

    xTs = [np.ascontiguousarray(inputs[b].T).astype(BFNP) for b in range(B)]
    xNs = [np.ascontiguousarray(inputs[b]).astype(BFNP) for b in range(B)]

    in_maps = []
    for core in range(NCORES):
        b, parity = core // 2, core % 2
        tm = _tile_map(parity)
        xT16 = xTs[b]
        cols = []
        for c in range(NCLS):
            for ht in range(2):
                r = tm[(c, ht)]
                cols.append(xT16[:, r * 128 : (r + 1) * 128])
        xq16 = np.ascontiguousarray(np.concatenate(cols, axis=1))
        in_maps.append(
            {
                "xT": xT16,
                "xN": xNs[b],
                "xq": xq16,
                "mT": mT16,
                "ct": cvec,
                "wvT": wvT,
                "bv": bv,
                "msk": msk_even if parity == 0 else msk_odd,
            }
        )

    nc = _build_program()
    res = None
    last_err = None
    for attempt in range(3):
        try:
            res = run_bass_kernel_spmd(nc, in_maps, list(range(NCORES)))
            break
        except Exception as e:  # transient NRT device wedge; retry
            last_err = e
            import time as _time

            _time.sleep(2.0)
    if res is None:
        raise last_err
    LAST_RESULT = res

    out = np.empty((B, S, H), dtype=np.float32)
    for core in range(NCORES):
        b, parity = core // 2, core % 2
        tm = _tile_map(parity)
        o = res.results[core]["out"]  # [8, 128, H]
        for c in range(NCLS):
            for ht in range(2):
                r = tm[(c, ht)]
                out[b, r * 128 : (r + 1) * 128, :] = o[2 * c + ht]
    return out
